# revision 61
# baseline (speedup 1.0000x reference)
"""Bahdanau additive attention kernel for 8 Trainium2 NeuronCores.

Data-parallel over batch: B=64 -> 8 batches per core. No collectives.

Per-batch math (reference):
  Wa   = dec @ Wa_w.T + Wa_b                       [1, H]
  Ua   = enc @ Ua_w.T + Ua_b                       [Te, H]
  s    = tanh(Ua + Wa) @ Va_w.T  (+ Va_b, dropped: softmax shift-invariant)
  w    = softmax(s)                                 [Te]
  ctx  = w @ enc                                    [1, De]

Device layout (per core, 8 batches):
  big matmul Ua:  out[t, h] = sum_d encT[d, t] * uawT[d, h]    (bf16, fp32 PSUM)
  bias:           VectorE add of WaPB broadcast (gpsimd partition_broadcast per b)
  tanh:           ScalarE, SBUF -> SBUF bf16
  score:          VectorE mult by Va bcast (bf16 2x mode) + reduce over h
                  -> scores as columns [128t, 8 chunks]
  softmax:        exp on ScalarE (scores bounded, no max subtraction),
                  normalization deferred to the end (unnormalized weights)
  context:        matmul, exp-weight column stationary vs encN [t, d] moving;
                  software-pipelined one batch behind the score stage
  scale:          ScalarE copy with scale = 1/sum(exp)

Cost-model timeline: 280.9us/core (TensorE busy 250.5us, within 2% of the
bf16 roofline for the 17.2 GFLOP/core; 45us startup-DMA + drain overhead).
Measured HW rel err 2.7e-3. Non-default options, all measured:
  KERNEL_UA_FP8=1  fp8e4m3+DoubleRow Ua matmul: 155us sim, HW-validated,
                   rel err 1.43e-2 (worst row 1.84e-2) - too close to a
                   2e-2 gate to ship by default.
  ctx_on="vector"  VectorE context reduction over resident encT: 1.9x
                   WORSE (partition-broadcast cost + serial chain).
Rejected by measurement: DMA splitting, prologue reordering/hoisting,
bias on TensorE, buffer-count increases, deferred encN prefetch.
"""

import os
import sys

import numpy as np
import ml_dtypes

for _p in ("/opt/trn_rl_repo",):
    if _p not in sys.path and os.path.isdir(_p):
        sys.path.append(_p)

import concourse.bass as bass
import concourse.tile as tile
import concourse.mybir as mybir
from concourse import bacc
from concourse.bass import ts
from concourse.bass_utils import run_bass_kernel_spmd
from concourse.masks import make_identity

B, T, D, H = 64, 1024, 1024, 1024
NCORES = 8
BPC = B // NCORES  # batches per core
P = 128
DC = D // P  # 8 contraction chunks
TC = T // P  # 8 t chunks

BF = mybir.dt.bfloat16
F8 = mybir.dt.float8e4
F32 = mybir.dt.float32
AF = mybir.ActivationFunctionType
ALU = mybir.AluOpType

# fp8e4m3 + DoubleRow for the Ua matmul (~1.5x TensorE); rel err ~1.4e-2 vs
# bf16's 2.7e-3 (gate 2e-2). Off unless KERNEL_UA_FP8=1.
UA_FP8 = bool(int(os.environ.get("KERNEL_UA_FP8", "0")))
# context matmul on "tensor" (TensorE, needs encN input) or "vector"
# (VectorE reduction over resident encT; drops the encN input entirely)
CTX_ON = os.environ.get("KERNEL_CTX", "tensor")
# run the two context d-halves concurrently in PE col-groups 0/64
CTX_COL2 = bool(int(os.environ.get("KERNEL_CTX_COL2", "1")))


def build_bass(
    bias_on: str = "vector",
    score_bf16: bool = True,
    pipelined: bool = True,
    enc_bufs: int = 2,
    work_bufs: int = 3,
    pu_bufs: int = 4,
    pc_bufs: int = 2,
    wb_via: str = "gpsimd",
    reduce_on: str = "vector",
    dma_split: int = 1,
    n_batches: int = BPC,
    ua_fp8: bool = UA_FP8,
    wapbrow_dma_on: str = "sync",
    hoist_first_enc: bool = False,
    ctx_on: str = "tensor",
    defer_nb0: bool = False,
    ctx_col2: bool = CTX_COL2,
):
    nc = bacc.Bacc("TRN2", target_bir_lowering=False, debug=False)

    va_dt = BF if score_bf16 else F32
    th_dt = BF if score_bf16 else F32
    enc_dt = F8 if ua_fp8 else BF
    assert not (ua_fp8 and ctx_on == "vector"), (
        "vector ctx reads EB; fp8 EB is too imprecise for the context reduction"
    )
    if ua_fp8:
        # DoubleRow psum group ends on the K=1 bias matmul; DVE-add path
        # would leave the group open across mixed perf modes.
        bias_on = "tensor"

    encT = nc.dram_tensor("encT", [BPC, D, T], enc_dt, kind="ExternalInput")
    encN = (
        nc.dram_tensor("encN", [BPC, T, D], BF, kind="ExternalInput")
        if ctx_on == "tensor"
        else None
    )
    uawT = nc.dram_tensor("uawT", [D, H], enc_dt, kind="ExternalInput")
    wawT = nc.dram_tensor("wawT", [D, H], BF, kind="ExternalInput")
    decT = nc.dram_tensor("decT", [D, BPC], BF, kind="ExternalInput")
    bsum = nc.dram_tensor("bsum", [1, H], BF, kind="ExternalInput")
    vabc = nc.dram_tensor("vabc", [P, H], va_dt, kind="ExternalInput")
    out = nc.dram_tensor("out", [BPC, D], F32, kind="ExternalOutput")

    with tile.TileContext(nc) as tc:
        with (
            tc.tile_pool(name="const", bufs=1) as cpool,
            tc.tile_pool(name="enc", bufs=enc_bufs) as epool,
            tc.tile_pool(name="work", bufs=work_bufs) as wpool,
            tc.tile_pool(name="pu", bufs=pu_bufs, space="PSUM") as pupool,
            tc.tile_pool(name="pc", bufs=pc_bufs, space="PSUM") as pcpool,
        ):
            def enc_dma(b, skip_nb_dma=False):
                EB = epool.tile([P, DC, T], enc_dt, tag="EB")
                srcT = encT.ap()[b].rearrange("(dc p) t -> p dc t", p=P)
                if ctx_on == "tensor":
                    NB = epool.tile([P, TC, D], BF, tag="NB")
                    srcN = encN.ap()[b].rearrange("(tc p) d -> p tc d", p=P)
                else:
                    NB = None
                split = dma_split if b == 0 else 1
                step = DC // split
                for s in range(split):
                    sl = slice(s * step, (s + 1) * step)
                    nc.sync.dma_start(EB[:, sl, :], srcT[:, sl, :])
                    if NB is not None and not skip_nb_dma:
                        nc.sync.dma_start(NB[:, sl, :], srcN[:, sl, :])
                return EB, NB

            def nb_dma(b, NB):
                srcN = encN.ap()[b].rearrange("(tc p) d -> p tc d", p=P)
                nc.sync.dma_start(NB[:], srcN)

            # batch-0 encoder tiles first: no deps, so the sync queue issues
            # them immediately and they overlap the weight DMAs
            enc0 = enc_dma(0) if hoist_first_enc else None

            # resident weights / constants
            UW = cpool.tile([P, DC, H], enc_dt, tag="UW")
            uw_src = uawT.ap().rearrange("(dc p) h -> p dc h", p=P)
            if dma_split > 1:
                for dc in range(DC):
                    nc.sync.dma_start(UW[:, dc : dc + 1, :], uw_src[:, dc : dc + 1, :])
            else:
                nc.sync.dma_start(UW[:], uw_src)
            WW = cpool.tile([P, DC, H], BF, tag="WW")
            nc.sync.dma_start(WW[:], wawT.ap().rearrange("(dc p) h -> p dc h", p=P))
            DT = cpool.tile([P, DC, BPC], BF, tag="DT")
            nc.sync.dma_start(DT[:], decT.ap().rearrange("(dc p) b -> p dc b", p=P))
            BS = cpool.tile([1, H], BF, tag="BS")
            nc.sync.dma_start(BS[:], bsum.ap())
            VAB = cpool.tile([P, H], va_dt, tag="VAB")
            nc.sync.dma_start(VAB[:], vabc.ap())

            ones_r = cpool.tile([1, P], BF, tag="ones_r")
            nc.vector.memset(ones_r[:], 1.0)
            ones_c = cpool.tile([P, 1], BF, tag="ones_c")
            nc.vector.memset(ones_c[:], 1.0)
            if ctx_on == "vector":
                IDN = cpool.tile([P, P], F32, tag="IDN")
                make_identity(nc, IDN[:])

            # WaPB[b, h] = dec_b @ Wa_w.T + (Wa_b + Ua_b), all batches at once,
            # then flattened to one partition so per-b rows are base-0 matmul rhs.
            WaPBs = cpool.tile([BPC, H], BF, tag="WaPBs")
            for hh in range(2):
                pw = pcpool.tile([BPC, 512], F32, tag="pc")
                for dc in range(DC):
                    nc.tensor.matmul(
                        pw[:],
                        DT[:, dc, :],
                        WW[:, dc, ts(hh, 512)],
                        start=(dc == 0),
                        stop=False,
                    )
                nc.tensor.matmul(
                    pw[:],
                    ones_r[:, 0:BPC],
                    BS[:, ts(hh, 512)],
                    start=False,
                    stop=True,
                )
                nc.vector.tensor_copy(WaPBs[:, ts(hh, 512)], pw[:])
            WaPBrow = cpool.tile([1, BPC * H], BF, tag="WaPBrow")
            # issue these row-flatten DMAs off the sync queue: they carry
            # semaphore waits on the WaPB copies and would head-of-line block
            # the encoder-tile DMAs queued behind them on sync
            wapb_dma = (
                nc.gpsimd.dma_start if wapbrow_dma_on == "gpsimd" else nc.sync.dma_start
            )
            for b in range(BPC):
                wapb_dma(WaPBrow[:, b * H : (b + 1) * H], WaPBs[b : b + 1, :])

            def scores_stage(b, pre=None):
                defer = defer_nb0 and b == 0
                EB, NB = pre if pre is not None else enc_dma(b, skip_nb_dma=defer)

                WaPB = WaPBrow[:, b * H : (b + 1) * H]
                if bias_on == "vector":
                    # broadcast WaPB to 128 partitions once per b
                    if wb_via == "gpsimd":
                        WB = wpool.tile([P, H], BF, tag="WB")
                        nc.gpsimd.partition_broadcast(WB[:], WaPB)
                    else:
                        WB = wpool.tile([P, H], F32, tag="WB")
                        for hh in range(2):
                            pb = pcpool.tile([P, 512], F32, tag="pb")
                            nc.tensor.matmul(
                                pb[:],
                                ones_r[:],
                                WaPB[:, ts(hh, 512)],
                                start=True,
                                stop=True,
                            )
                            nc.vector.tensor_copy(WB[:, ts(hh, 512)], pb[:])
                SC = wpool.tile([P, TC], F32, tag="SC")
                for tci in range(TC):
                    pu0 = pupool.tile([P, 512], F32, tag="pu")
                    pu1 = pupool.tile([P, 512], F32, tag="pu")
                    last = bias_on != "tensor"
                    if ua_fp8:
                        # DoubleRow: contract two 128-chunks per matmul via
                        # 3D APs [128, 2, M] / [128, 2, N]
                        for dc in range(0, DC, 2):
                            lh = EB[:, dc : dc + 2, ts(tci, P)]
                            nc.tensor.matmul(
                                pu0[:],
                                lh,
                                UW[:, dc : dc + 2, 0:512],
                                start=(dc == 0),
                                stop=False,
                                perf_mode=mybir.MatmulPerfMode.DoubleRow,
                            )
                            nc.tensor.matmul(
                                pu1[:],
                                lh,
                                UW[:, dc : dc + 2, 512:1024],
                                start=(dc == 0),
                                stop=False,
                                perf_mode=mybir.MatmulPerfMode.DoubleRow,
                            )
                    else:
                        for dc in range(DC):
                            lh = EB[:, dc, ts(tci, P)]
                            nc.tensor.matmul(
                                pu0[:],
                                lh,
                                UW[:, dc, 0:512],
                                start=(dc == 0),
                                stop=(last and dc == DC - 1),
                            )
                            nc.tensor.matmul(
                                pu1[:],
                                lh,
                                UW[:, dc, 512:1024],
                                start=(dc == 0),
                                stop=(last and dc == DC - 1),
                            )
                    TH = wpool.tile([P, H], th_dt, tag="TH")
                    if bias_on == "tensor":
                        # += WaPB broadcast along t partitions (K=1 ones matmul)
                        nc.tensor.matmul(
                            pu0[:], ones_r[:], WaPB[:, 0:512], start=False, stop=True
                        )
                        nc.tensor.matmul(
                            pu1[:], ones_r[:], WaPB[:, 512:1024], start=False, stop=True
                        )
                        nc.scalar.activation(TH[:, 0:512], pu0[:], AF.Tanh)
                        nc.scalar.activation(TH[:, 512:1024], pu1[:], AF.Tanh)
                    else:
                        T1 = wpool.tile([P, H], F32, tag="T1")
                        nc.vector.tensor_tensor(
                            T1[:, 0:512], pu0[:], WB[:, 0:512], ALU.add
                        )
                        nc.vector.tensor_tensor(
                            T1[:, 512:1024], pu1[:], WB[:, 512:1024], ALU.add
                        )
                        nc.scalar.activation(TH[:, 0:512], T1[:, 0:512], AF.Tanh)
                        nc.scalar.activation(TH[:, 512:1024], T1[:, 512:1024], AF.Tanh)
                    TMP = wpool.tile([P, H], th_dt, tag="TMP")
                    nc.vector.tensor_tensor(TMP[:], TH[:], VAB[:], ALU.mult)
                    if reduce_on == "scalar":
                        TJ = wpool.tile([P, H], th_dt, tag="TJ")
                        nc.scalar.activation(
                            TJ[:],
                            TMP[:],
                            AF.Identity,
                            accum_out=SC[:, tci : tci + 1],
                        )
                    else:
                        nc.vector.tensor_reduce(
                            SC[:, tci : tci + 1],
                            TMP[:],
                            axis=mybir.AxisListType.X,
                            op=ALU.add,
                        )
                if defer and NB is not None:
                    nb_dma(b, NB)
                return SC, NB, EB

            def ctx_stage(b, SC, NB, EB):
                if ctx_on == "vector":
                    return ctx_stage_vector(b, SC, EB)
                # unnormalized softmax weights, bf16 columns [128t, TC]
                EW = wpool.tile([P, TC], BF, tag="EW")
                nc.scalar.activation(EW[:], SC[:], AF.Exp)
                psum_s = pcpool.tile([1, TC], F32, tag="pc")
                nc.tensor.matmul(psum_s[:], ones_c[:], EW[:], start=True, stop=True)
                TOT = wpool.tile([1, 1], F32, tag="TOT")
                nc.vector.tensor_reduce(
                    TOT[:], psum_s[:], axis=mybir.AxisListType.X, op=ALU.add
                )
                INV = wpool.tile([1, 1], F32, tag="INV")
                nc.vector.reciprocal(INV[:], TOT[:])

                if ctx_col2:
                    # run the two d-halves concurrently in PE col-groups 0 and
                    # 64 (tile_position): M=1 uses 1/128 of the array, so the
                    # two matmul chains overlap on HW (~2x ctx speedup; the
                    # cost model prices them serially). One shared PSUM bank,
                    # rows 0 and 64; only the first matmul may carry
                    # start=True — it clears has_written for the whole bank.
                    INV128 = wpool.tile([P, 1], F32, tag="INV128")
                    nc.gpsimd.partition_broadcast(INV128[:], INV[:])
                    pts = [
                        pcpool.tile([P, 512], F32, tag="pc", name=f"pt{b}_0"),
                        pcpool.tile([P, 512], F32, tag="pc", name=f"pt{b}_1"),
                    ]
                    for tci in range(TC):
                        for j, dh in ((0, 0), (64, 1)):
                            nc.tensor.matmul(
                                pts[dh][j : j + 1, :],
                                EW[:, tci : tci + 1],
                                NB[:, tci, ts(dh, 512)],
                                start=(tci == 0),
                                stop=(tci == TC - 1),
                                tile_position=(0, j),
                            )
                    OUTx = wpool.tile([P, 512], F32, tag="OUTx")
                    for j, dh in ((0, 0), (64, 1)):
                        nc.scalar.activation(
                            OUTx[j : j + 1, :],
                            pts[dh][j : j + 1, :],
                            AF.Copy,
                            scale=INV128[j : j + 1],
                        )
                        nc.sync.dma_start(
                            out.ap()[b : b + 1, ts(dh, 512)], OUTx[j : j + 1, :]
                        )
                else:
                    OUTb = wpool.tile([1, D], F32, tag="OUTb")
                    for dh in range(2):
                        pc = pcpool.tile([1, 512], F32, tag="pc")
                        for tci in range(TC):
                            nc.tensor.matmul(
                                pc[:],
                                EW[:, tci : tci + 1],
                                NB[:, tci, ts(dh, 512)],
                                start=(tci == 0),
                                stop=(tci == TC - 1),
                            )
                        nc.scalar.activation(
                            OUTb[:, ts(dh, 512)], pc[:], AF.Copy, scale=INV[:]
                        )
                    nc.sync.dma_start(out.ap()[b : b + 1, :], OUTb[:])

            def ctx_stage_vector(b, SC, EB):
                # scores columns [128t', TC] -> one row [1, T] via PE transpose
                # + flatten DMAs, so exp/softmax-sum run on a single ACT op and
                # the weights can be partition-broadcast for the VectorE
                # context reduction over the already-resident encT tiles.
                pt = pcpool.tile([TC, P], F32, tag="pc")
                nc.tensor.transpose(pt[:], SC[:], IDN[:])
                SROW8 = wpool.tile([TC, P], F32, tag="SROW8")
                nc.vector.tensor_copy(SROW8[:], pt[:])
                SROWf = wpool.tile([1, T], F32, tag="SROWf")
                for tci in range(TC):
                    nc.sync.dma_start(
                        SROWf[:, ts(tci, P)], SROW8[tci : tci + 1, :]
                    )
                EWrow = wpool.tile([1, T], BF, tag="EWrow")
                TOT = wpool.tile([1, 1], F32, tag="TOT")
                nc.scalar.activation(EWrow[:], SROWf[:], AF.Exp, accum_out=TOT[:])
                INV = wpool.tile([1, 1], F32, tag="INV")
                nc.vector.reciprocal(INV[:], TOT[:])
                INV128 = wpool.tile([P, 1], F32, tag="INV128")
                nc.gpsimd.partition_broadcast(INV128[:], INV[:])
                EWbc = wpool.tile([P, T], BF, tag="EWbc")
                nc.gpsimd.partition_broadcast(EWbc[:], EWrow[:])

                CTXc = wpool.tile([P, DC], F32, tag="CTXc")
                for dc in range(DC):
                    TMP2 = wpool.tile([P, T], BF, tag="TMP")
                    nc.vector.tensor_tensor(TMP2[:], EB[:, dc, :], EWbc[:], ALU.mult)
                    nc.vector.tensor_reduce(
                        CTXc[:, dc : dc + 1],
                        TMP2[:],
                        axis=mybir.AxisListType.X,
                        op=ALU.add,
                    )
                nc.vector.tensor_scalar_mul(CTXc[:], CTXc[:], INV128[:])
                nc.sync.dma_start(
                    out.ap()[b].rearrange("(dc p) -> p dc", p=P), CTXc[:]
                )

            if pipelined:
                prev = None
                for b in range(n_batches):
                    cur = scores_stage(b, pre=enc0 if b == 0 else None)
                    if prev is not None:
                        ctx_stage(b - 1, *prev)
                    prev = cur
                ctx_stage(n_batches - 1, *prev)
            else:
                for b in range(n_batches):
                    SC, NB = scores_stage(b, pre=enc0 if b == 0 else None)
                    ctx_stage(b, SC, NB)

    nc.finalize()
    return nc


_NC = None


def _get_nc():
    global _NC
    if _NC is None:
        _NC = build_bass(ctx_on=CTX_ON)
    return _NC


LAST_RESULTS = None


def prepare_in_maps(inputs, ua_fp8: bool = UA_FP8, ctx_on: str = CTX_ON) -> list:
    enc = np.asarray(inputs["encoder_outputs"], dtype=np.float32)  # [B, T, D]
    dec = np.asarray(inputs["decoder_outputs"], dtype=np.float32)[:, 0, :]  # [B, D]
    Wa_w = np.asarray(inputs["Wa_w"], dtype=np.float32)
    Wa_b = np.asarray(inputs["Wa_b"], dtype=np.float32)
    Ua_w = np.asarray(inputs["Ua_w"], dtype=np.float32)
    Ua_b = np.asarray(inputs["Ua_b"], dtype=np.float32)
    Va_w = np.asarray(inputs["Va_w"], dtype=np.float32)
    # Va_b dropped: softmax(s + c) == softmax(s)

    bf16 = ml_dtypes.bfloat16
    enc_t_dt = ml_dtypes.float8_e4m3 if ua_fp8 else bf16
    enc_bf = enc.astype(bf16)  # [B, T, D]
    encN_all = enc_bf.reshape(NCORES, BPC, T, D)
    encT_all = (
        np.ascontiguousarray(enc.transpose(0, 2, 1))
        .astype(enc_t_dt)
        .reshape(NCORES, BPC, D, T)
    )
    decT_all = np.ascontiguousarray(
        dec.reshape(NCORES, BPC, D).transpose(0, 2, 1)
    ).astype(bf16)  # [NCORES, D, BPC]
    uawT = np.ascontiguousarray(Ua_w.T).astype(enc_t_dt)
    wawT = np.ascontiguousarray(Wa_w.T).astype(bf16)
    bsum = (Wa_b + Ua_b).reshape(1, H).astype(bf16)
    vabc = np.ascontiguousarray(np.broadcast_to(Va_w.reshape(1, H), (P, H))).astype(
        bf16
    )

    maps = [
        {
            "encT": np.ascontiguousarray(encT_all[c]),
            "uawT": uawT,
            "wawT": wawT,
            "decT": np.ascontiguousarray(decT_all[c]),
            "bsum": bsum,
            "vabc": vabc,
        }
        for c in range(NCORES)
    ]
    if ctx_on == "tensor":
        for c in range(NCORES):
            maps[c]["encN"] = np.ascontiguousarray(encN_all[c])
    return maps


def kernel(**inputs) -> np.ndarray:
    in_maps = prepare_in_maps(inputs)
    nc = _get_nc()
    trace = bool(int(os.environ.get("KERNEL_TRACE", "0")))
    try:
        res = run_bass_kernel_spmd(
            nc, in_maps, core_ids=list(range(NCORES)), trace=trace
        )
    except ModuleNotFoundError:
        # axon clients without the NTFF hook (antenv.axon_hooks) cannot trace;
        # retry untraced rather than failing the whole run
        os.environ["BASS_NEVER_TRACE"] = "1"
        res = run_bass_kernel_spmd(
            nc, in_maps, core_ids=list(range(NCORES)), trace=False
        )
    global LAST_RESULTS
    LAST_RESULTS = res

    outs = [res.results[c]["out"] for c in range(NCORES)]
    full = np.concatenate(outs, axis=0).reshape(B, 1, D).astype(np.float32)
    return full
